# revision 24
# baseline (speedup 1.0000x reference)
"""Block-sparse MoE (SwiGLU, top-k of 8 experts) on 8 Trainium2 NeuronCores.

Sharding: expert-parallel with host-side all-to-all token dispatch.
  - Routing (gate matmul + softmax + top-k, ~0.07% of total FLOPs) runs on
    the host; each token is dispatched to its top-k experts.
  - Core e runs the SwiGLU FFN of expert e over only the tokens routed to
    it (padded to a uniform capacity C), pre-scaled by the routing weight.
  - The host scatter-adds the per-expert outputs into the full [T, H] out.

Device kernel per core (capacity C tokens, H=2048, F=4096):
  phase 1: interT[f, c] = silu(w1 @ xT) * (w3 @ xT)   (PSUM-accumulated over H)
  phase 2: yT[h, c]     = (w2.T @ interT) * wgt[c]    (PSUM-accumulated over F)
Matmuls run as float32r (fp32 data, full PE rate at moving-dim >= 256).
All DRAM->SBUF transfers are host-pre-tiled so every DMA is contiguous.
"""

import math
import os

import numpy as np

H = 2048          # hidden dim
F = 4096          # ffn dim per expert
E = 8             # experts
NCORES = 8
P = 128           # partitions
NH = H // P       # 16 h-tiles
NF = F // P       # 32 f-tiles

DT_MODE = os.environ.get("MOE_DT", "f32r")   # f32r | bf16 | f32

# populated by kernel() for test harness introspection
LAST_STATS = {}

_BUILD_CACHE = {}


def _chunks(total, cmax=512):
    """Split `total` into near-equal EVEN-width chunks each <= cmax
    (>=256 when total>=256; fp32r matmuls require an even moving dim)."""
    assert total % 2 == 0, total
    n = max(1, math.ceil(total / cmax))
    w0 = 2 * math.ceil(total / (2 * n))
    out = []
    pos = 0
    for i in range(n):
        w = min(w0, total - pos)
        out.append((pos, w))
        pos += w
    assert pos == total and all(w % 2 == 0 for _, w in out)
    return out


def _build(C, dt_mode, h=H, f=F):
    """Build + compile the per-core Bass program for capacity C."""
    import concourse.bacc as bacc
    import concourse.mybir as mybir
    from concourse import tile

    AF = mybir.ActivationFunctionType
    f32 = mybir.dt.float32
    if dt_mode == "bf16":
        dmm = mybir.dt.bfloat16    # dtype of matmul operands as stored
    elif dt_mode == "fp16":
        dmm = mybir.dt.float16
    elif dt_mode == "f32":
        dmm = f32
    else:
        # float32r: fp32 bytes, reduced-precision multiply at full PE rate.
        # The BIR verifier requires every producer of an fp32r matmul
        # operand to emit float32r, so the whole operand chain is f32r.
        dmm = mybir.dt.float32r

    nh, nf = h // P, f // P
    chunks = _chunks(C)

    def mm_ap(ap):
        return ap

    nc = bacc.Bacc("TRN2", target_bir_lowering=False, debug=False)

    # Host-pre-tiled DRAM layouts (every DMA below is fully contiguous):
    #   xt   [P, nh, C]        xt[p, n, c]        = x_tok[c, n*P + p]
    #   w13t [nf, P, 2, nh, P] w13t[fi, p, m, n, j] = w{1,3}[fi*P+j, n*P+p]
    #   w2t  [nh, P, nf, P]    w2t[ht, p, fi, j]  = w2[fi*P + p, ht*P + j]
    #   wgtb [P, C]            broadcast routing weights
    #   yt   [h, C]            output, yt[h, c]
    xt_d = nc.dram_tensor("xt", [P, nh, C], dmm, kind="ExternalInput").ap()
    w13_d = nc.dram_tensor("w13t", [nf, P, 2, nh, P], dmm,
                           kind="ExternalInput").ap()
    w2_d = nc.dram_tensor("w2t", [nh, P, nf, P], dmm, kind="ExternalInput").ap()
    wg_d = nc.dram_tensor("wgtb", [P, C], f32, kind="ExternalInput").ap()
    y_d = nc.dram_tensor("yt", [h, C], f32, kind="ExternalOutput").ap()

    with tile.TileContext(nc) as tc:
        with (
            tc.tile_pool(name="inter", bufs=1) as inter_pool,
            tc.tile_pool(name="psum", bufs=2, space="PSUM") as psum_pool,
            tc.tile_pool(name="misc", bufs=1) as misc_pool,
            tc.tile_pool(name="xtp", bufs=1) as xt_pool,
            tc.tile_pool(name="wcol", bufs=2) as wcol_pool,
            tc.tile_pool(name="p1tmp", bufs=2) as p1tmp,
            tc.tile_pool(name="w2col", bufs=2) as w2_pool,
            tc.tile_pool(name="p2tmp", bufs=2) as p2tmp,
        ):
            # xt split per chunk, spread across DMA rings, so the first
            # matmuls start as early as possible (a single big load added
            # a ~26 us PE startup stall)
            xt_rings = [nc.scalar, nc.gpsimd]
            xt_tiles = []
            for ci, (c0, cw) in enumerate(chunks):
                xtc = xt_pool.tile([P, nh, cw], dmm, tag=f"xt{ci}",
                                   name=f"xt{ci}")
                xt_rings[ci % len(xt_rings)].dma_start(
                    xtc[:], xt_d[:, :, c0:c0 + cw])
                xt_tiles.append(xtc)

            wgtb = misc_pool.tile([P, C], f32, tag="wgtb")
            nc.gpsimd.dma_start(wgtb[:], wg_d[:])

            inter_tiles = []
            nch = len(chunks)

            # ---- phase 1: interT[f, :] = silu(w1 @ xT) * (w3 @ xT) ----
            # weight columns stream on the SP HWDGE ring (one 2 MB DMA/fi).
            # chunks are interleaved per h-tile so consecutive matmuls share
            # the same stationary operand.
            wc_dmas = []
            for fi in range(nf):
                wc = wcol_pool.tile([P, 2, nh, P], dmm, tag="wc")
                wc_dmas.append(nc.sync.dma_start(wc[:], w13_d[fi]))
                it = inter_pool.tile([P, C], dmm, tag=f"inter{fi}",
                                     name=f"inter{fi}")
                inter_tiles.append(it)
                if nch <= 2:
                    # interleaved: consecutive matmuls share the stationary
                    ps1 = [psum_pool.tile([P, cw], f32, tag="ps1", bufs=3,
                                          name=f"ps1_{fi}_{ci}")
                           for ci, (c0, cw) in enumerate(chunks)]
                    ps3 = [psum_pool.tile([P, cw], f32, tag="ps3", bufs=3,
                                          name=f"ps3_{fi}_{ci}")
                           for ci, (c0, cw) in enumerate(chunks)]
                    for hi in range(nh):
                        for ci in range(nch):
                            nc.tensor.matmul(
                                ps1[ci][:], wc[:, 0, hi, :],
                                xt_tiles[ci][:, hi, :],
                                start=(hi == 0), stop=(hi == nh - 1))
                        for ci in range(nch):
                            nc.tensor.matmul(
                                ps3[ci][:], wc[:, 1, hi, :],
                                xt_tiles[ci][:, hi, :],
                                start=(hi == 0), stop=(hi == nh - 1))
                    for ci, (c0, cw) in enumerate(chunks):
                        # silu(a) = a * sigmoid(a)
                        sig = p1tmp.tile([P, cw], f32, tag="sig")
                        nc.scalar.activation(sig[:], ps1[ci][:], AF.Sigmoid)
                        sil = p1tmp.tile([P, cw], f32, tag="sil")
                        nc.vector.tensor_mul(sil[:], ps1[ci][:], sig[:])
                        nc.vector.tensor_mul(it[:, c0:c0 + cw], sil[:],
                                             ps3[ci][:])
                else:
                    for ci, (c0, cw) in enumerate(chunks):
                        ps1s = psum_pool.tile([P, cw], f32, tag="ps1")
                        ps3s = psum_pool.tile([P, cw], f32, tag="ps3")
                        for hi in range(nh):
                            nc.tensor.matmul(
                                ps1s[:], wc[:, 0, hi, :],
                                xt_tiles[ci][:, hi, :],
                                start=(hi == 0), stop=(hi == nh - 1))
                        for hi in range(nh):
                            nc.tensor.matmul(
                                ps3s[:], wc[:, 1, hi, :],
                                xt_tiles[ci][:, hi, :],
                                start=(hi == 0), stop=(hi == nh - 1))
                        sig = p1tmp.tile([P, cw], f32, tag="sig")
                        nc.scalar.activation(sig[:], ps1s[:], AF.Sigmoid)
                        sil = p1tmp.tile([P, cw], f32, tag="sil")
                        nc.vector.tensor_mul(sil[:], ps1s[:], sig[:])
                        nc.vector.tensor_mul(it[:, c0:c0 + cw], sil[:],
                                             ps3s[:])

            # ---- phase 2: yT[ht, :] = (w2.T @ interT) * wgt ----
            # w2 columns stream on the ACT HWDGE ring; output on SWDGE
            for ht in range(nh):
                w2c = w2_pool.tile([P, nf, P], dmm, tag="w2c")
                w2c_dma = nc.scalar.dma_start(w2c[:], w2_d[ht])
                if ht < 2:
                    # keep the prefetch off the HBM-saturated startup window
                    tile.add_dep_helper(
                        w2c_dma.ins, wc_dmas[nf // 3].ins,
                        reason="delay w2 prefetch past kernel startup")
                ob = p2tmp.tile([P, C], f32, tag="ob")
                if nch <= 2:
                    po = [psum_pool.tile([P, cw], f32, tag="po", bufs=2,
                                         name=f"po_{ht}_{ci}")
                          for ci, (c0, cw) in enumerate(chunks)]
                    for fi in range(nf):
                        for ci in range(nch):
                            nc.tensor.matmul(
                                po[ci][:], w2c[:, fi, :],
                                inter_tiles[fi][:, chunks[ci][0]:
                                                chunks[ci][0] + chunks[ci][1]],
                                start=(fi == 0), stop=(fi == nf - 1))
                    for ci, (c0, cw) in enumerate(chunks):
                        nc.vector.tensor_mul(ob[:, c0:c0 + cw], po[ci][:],
                                             wgtb[:, c0:c0 + cw])
                else:
                    for ci, (c0, cw) in enumerate(chunks):
                        pos = psum_pool.tile([P, cw], f32, tag="po")
                        for fi in range(nf):
                            nc.tensor.matmul(
                                pos[:], w2c[:, fi, :],
                                inter_tiles[fi][:, c0:c0 + cw],
                                start=(fi == 0), stop=(fi == nf - 1))
                        nc.vector.tensor_mul(ob[:, c0:c0 + cw], pos[:],
                                             wgtb[:, c0:c0 + cw])
                nc.gpsimd.dma_start(y_d[ht * P:(ht + 1) * P, :], ob[:])

    nc.compile()
    return nc


def _get_nc(C, dt_mode):
    key = (C, dt_mode)
    if key not in _BUILD_CACHE:
        _BUILD_CACHE[key] = _build(C, dt_mode)
    return _BUILD_CACHE[key]


def _route(x, gate_w, top_k):
    """Host routing, matching the reference exactly:
    softmax(x @ gate_w.T) -> top-k (ties -> lower index) -> renormalize."""
    t = x.shape[0]
    logits = x.astype(np.float64) @ gate_w.astype(np.float64).T
    m = logits.max(axis=-1, keepdims=True)
    p = np.exp(logits - m)
    p /= p.sum(axis=-1, keepdims=True)
    idx = np.argsort(-p, axis=-1, kind="stable")[:, :top_k]          # [T, k]
    vals = np.take_along_axis(p, idx, axis=-1)
    vals = vals / vals.sum(axis=-1, keepdims=True)
    return idx, vals.astype(np.float32)


def _fake_device(in_maps):
    """Numpy stand-in for the device: consumes the exact tiled in_maps
    (validates host-side layouts end-to-end). Dev aid, off by default."""
    class R:
        exec_time_ns = None
        mean_exec_time_ns = None
        results = []
    res = R()
    for m in in_maps:
        C = m["xt"].shape[2]
        xs = m["xt"].transpose(1, 0, 2).reshape(H, C).T.astype(np.float32)
        w1e = m["w13t"][:, :, 0].transpose(0, 3, 2, 1).reshape(F, H).astype(
            np.float32)
        w3e = m["w13t"][:, :, 1].transpose(0, 3, 2, 1).reshape(F, H).astype(
            np.float32)
        w2e = m["w2t"].transpose(2, 1, 0, 3).reshape(F, H).astype(np.float32)
        wgt = m["wgtb"][0]
        h1 = xs @ w1e.T
        h3 = xs @ w3e.T
        inter = (h1 / (1 + np.exp(-h1))) * h3
        y = (inter @ w2e) * wgt[:, None]
        res.results.append({"yt": np.ascontiguousarray(y.T)})
    return res


def kernel(x, gate_w, w1, w2, w3, top_k):
    from concourse.bass_utils import run_bass_kernel_spmd

    x = np.ascontiguousarray(np.asarray(x, dtype=np.float32))
    gate_w = np.asarray(gate_w, dtype=np.float32)
    w1 = np.asarray(w1, dtype=np.float32)
    w2 = np.asarray(w2, dtype=np.float32)
    w3 = np.asarray(w3, dtype=np.float32)
    k = int(np.asarray(top_k))
    t, h = x.shape
    e = gate_w.shape[0]
    f = w1.shape[0] // e
    assert (h, f, e) == (H, F, E), (h, f, e)

    dt_mode = DT_MODE
    import ml_dtypes
    np_mm = {"bf16": ml_dtypes.bfloat16, "fp16": np.float16}.get(
        dt_mode, np.float32)

    idx, vals = _route(x, gate_w, k)                                  # [T, k]

    # token lists per expert
    tok_lists = []
    wgt_lists = []
    for ei in range(E):
        tok_i, slot_i = np.nonzero(idx == ei)
        tok_lists.append(tok_i.astype(np.int64))
        wgt_lists.append(vals[tok_i, slot_i].astype(np.float32))
    max_count = max(len(ti) for ti in tok_lists)
    C = max(256, ((max_count + 3) // 4) * 4)

    xmm = x.astype(np_mm)
    in_maps = []
    for ei in range(E):
        tok = tok_lists[ei]
        n = len(tok)
        xs = np.zeros((C, H), dtype=np_mm)
        xs[:n] = xmm[tok]
        # xt [P, NH, C]
        xt = np.ascontiguousarray(xs.T.reshape(NH, P, C).transpose(1, 0, 2))
        w1e = w1[ei * F:(ei + 1) * F].astype(np_mm)
        w3e = w3[ei * F:(ei + 1) * F].astype(np_mm)
        w2e = w2[ei * F:(ei + 1) * F].astype(np_mm)
        # w13t [NF, P, 2, NH, P]: [fi, p, m, n, j] = w{1,3}[fi*P+j, n*P+p]
        w13t = np.ascontiguousarray(np.stack(
            [w1e.reshape(NF, P, NH, P).transpose(0, 3, 2, 1),
             w3e.reshape(NF, P, NH, P).transpose(0, 3, 2, 1)], axis=2))
        # w2t [NH, P, NF, P]: [ht, p, fi, j] = w2[fi*P+p, ht*P+j]
        w2t = np.ascontiguousarray(
            w2e.reshape(NF, P, NH, P).transpose(2, 1, 0, 3))
        wgt = np.zeros(C, dtype=np.float32)
        wgt[:n] = wgt_lists[ei]
        wgtb = np.ascontiguousarray(
            np.broadcast_to(wgt, (P, C)).astype(np.float32))
        in_maps.append({
            "xt": xt, "w13t": w13t, "w2t": w2t, "wgtb": wgtb,
        })

    if os.environ.get("MOE_FAKE"):
        res = _fake_device(in_maps)
    else:
        nc = _get_nc(C, dt_mode)
        trace = bool(int(os.environ.get("MOE_TRACE", "0")))
        res = run_bass_kernel_spmd(nc, in_maps, core_ids=list(range(NCORES)),
                                   trace=trace)
    LAST_STATS.clear()
    LAST_STATS.update({
        "C": C,
        "dt_mode": dt_mode,
        "exec_time_ns": res.exec_time_ns,
        "mean_exec_time_ns": res.mean_exec_time_ns,
        "counts": [len(ti) for ti in tok_lists],
    })

    out = np.zeros((t, h), dtype=np.float32)
    for ei in range(E):
        n = len(tok_lists[ei])
        yt = res.results[ei]["yt"]                                    # [H, C]
        out[tok_lists[ei]] += yt[:, :n].T
    return out


# revision 27
# speedup vs baseline: 1.1717x; 1.1717x over previous
"""Block-sparse MoE (SwiGLU, top-k of 8 experts) on 8 Trainium2 NeuronCores.

Sharding: expert-parallel with host-side all-to-all token dispatch.
  - Routing (gate matmul + softmax + top-k, ~0.07% of total FLOPs) runs on
    the host; each token is dispatched to its top-k experts.
  - Core e runs the SwiGLU FFN of expert e over only the tokens routed to
    it (padded to a uniform capacity C), pre-scaled by the routing weight.
  - The host scatter-adds the per-expert outputs into the full [T, H] out.

Device kernel per core (capacity C tokens, H=2048, F=4096):
  phase 1: interT[f, c] = silu(w1 @ xT) * (w3 @ xT)   (PSUM-accumulated over H)
  phase 2: yT[h, c]     = (w2.T @ interT) * wgt[c]    (PSUM-accumulated over F)
Matmuls run as float32r (fp32 data, full PE rate at moving-dim >= 256).
All DRAM->SBUF transfers are host-pre-tiled so every DMA is contiguous.
"""

import math
import os

import numpy as np

H = 2048          # hidden dim
F = 4096          # ffn dim per expert
E = 8             # experts
NCORES = 8
P = 128           # partitions
NH = H // P       # 16 h-tiles
NF = F // P       # 32 f-tiles

DT_MODE = os.environ.get("MOE_DT", "fp16")   # fp16 | f32r | bf16 | f32

# populated by kernel() for test harness introspection
LAST_STATS = {}

_BUILD_CACHE = {}


def _chunks(total, cmax=512):
    """Split `total` into near-equal EVEN-width chunks each <= cmax
    (>=256 when total>=256; fp32r matmuls require an even moving dim)."""
    assert total % 2 == 0, total
    n = max(1, math.ceil(total / cmax))
    w0 = 2 * math.ceil(total / (2 * n))
    out = []
    pos = 0
    for i in range(n):
        w = min(w0, total - pos)
        out.append((pos, w))
        pos += w
    assert pos == total and all(w % 2 == 0 for _, w in out)
    return out


def _build(C, dt_mode, h=H, f=F):
    """Build + compile the per-core Bass program for capacity C."""
    import concourse.bacc as bacc
    import concourse.mybir as mybir
    from concourse import tile

    AF = mybir.ActivationFunctionType
    f32 = mybir.dt.float32
    if dt_mode == "bf16":
        dmm = mybir.dt.bfloat16    # dtype of matmul operands as stored
    elif dt_mode == "fp16":
        dmm = mybir.dt.float16
    elif dt_mode == "f32":
        dmm = f32
    else:
        # float32r: fp32 bytes, reduced-precision multiply at full PE rate.
        # The BIR verifier requires every producer of an fp32r matmul
        # operand to emit float32r, so the whole operand chain is f32r.
        dmm = mybir.dt.float32r

    nh, nf = h // P, f // P
    chunks = _chunks(C)

    def mm_ap(ap):
        return ap

    nc = bacc.Bacc("TRN2", target_bir_lowering=False, debug=False)

    # Host-pre-tiled DRAM layouts (every DMA below is fully contiguous):
    #   xt   [P, nh, C]        xt[p, n, c]        = x_tok[c, n*P + p]
    #   w13t [nf, P, 2, nh, P] w13t[fi, p, m, n, j] = w{1,3}[fi*P+j, n*P+p]
    #   w2t  [nh, P, nf, P]    w2t[ht, p, fi, j]  = w2[fi*P + p, ht*P + j]
    #   wgtb [P, C]            broadcast routing weights
    #   yt   [h, C]            output, yt[h, c]
    xt_d = nc.dram_tensor("xt", [P, nh, C], dmm, kind="ExternalInput").ap()
    w13_d = nc.dram_tensor("w13t", [nf, P, 2, nh, P], dmm,
                           kind="ExternalInput").ap()
    w2_d = nc.dram_tensor("w2t", [nh, P, nf, P], dmm, kind="ExternalInput").ap()
    wg_d = nc.dram_tensor("wgtb", [P, C], f32, kind="ExternalInput").ap()
    y_d = nc.dram_tensor("yt", [h, C], f32, kind="ExternalOutput").ap()

    with tile.TileContext(nc) as tc:
        with (
            tc.tile_pool(name="inter", bufs=1) as inter_pool,
            tc.tile_pool(name="psum", bufs=2, space="PSUM") as psum_pool,
            tc.tile_pool(name="misc", bufs=1) as misc_pool,
            tc.tile_pool(name="xtp", bufs=1) as xt_pool,
            tc.tile_pool(name="wcol", bufs=2) as wcol_pool,
            tc.tile_pool(name="p1tmp", bufs=2) as p1tmp,
            tc.tile_pool(name="w2col", bufs=2) as w2_pool,
            tc.tile_pool(name="p2tmp", bufs=2) as p2tmp,
        ):
            # xt split per chunk, spread across DMA rings, so the first
            # matmuls start as early as possible (a single big load added
            # a ~26 us PE startup stall)
            xt_rings = [nc.scalar]
            xt_tiles = []
            for ci, (c0, cw) in enumerate(chunks):
                xtc = xt_pool.tile([P, nh, cw], dmm, tag=f"xt{ci}",
                                   name=f"xt{ci}")
                xt_rings[ci % len(xt_rings)].dma_start(
                    xtc[:], xt_d[:, :, c0:c0 + cw])
                xt_tiles.append(xtc)

            wgtb = misc_pool.tile([P, C], f32, tag="wgtb")
            nc.gpsimd.dma_start(wgtb[:], wg_d[:])

            inter_tiles = []
            nch = len(chunks)

            # ---- phase 1: interT[f, :] = silu(w1 @ xT) * (w3 @ xT) ----
            # weight columns stream on the SP HWDGE ring (one 2 MB DMA/fi).
            # chunks are interleaved per h-tile so consecutive matmuls share
            # the same stationary operand.
            wc_dmas = []
            for fi in range(nf):
                wc = wcol_pool.tile([P, 2, nh, P], dmm, tag="wc")
                wc_dmas.append(nc.sync.dma_start(wc[:], w13_d[fi]))
                it = inter_pool.tile([P, C], dmm, tag=f"inter{fi}",
                                     name=f"inter{fi}")
                inter_tiles.append(it)
                if nch <= 2:
                    # interleaved: consecutive matmuls share the stationary
                    ps1 = [psum_pool.tile([P, cw], f32, tag="ps1", bufs=3,
                                          name=f"ps1_{fi}_{ci}")
                           for ci, (c0, cw) in enumerate(chunks)]
                    ps3 = [psum_pool.tile([P, cw], f32, tag="ps3", bufs=3,
                                          name=f"ps3_{fi}_{ci}")
                           for ci, (c0, cw) in enumerate(chunks)]
                    for hi in range(nh):
                        for ci in range(nch):
                            nc.tensor.matmul(
                                ps1[ci][:], wc[:, 0, hi, :],
                                xt_tiles[ci][:, hi, :],
                                start=(hi == 0), stop=(hi == nh - 1))
                        for ci in range(nch):
                            nc.tensor.matmul(
                                ps3[ci][:], wc[:, 1, hi, :],
                                xt_tiles[ci][:, hi, :],
                                start=(hi == 0), stop=(hi == nh - 1))
                    for ci, (c0, cw) in enumerate(chunks):
                        # silu(a) = a * sigmoid(a)
                        sig = p1tmp.tile([P, cw], f32, tag="sig")
                        nc.scalar.activation(sig[:], ps1[ci][:], AF.Sigmoid)
                        sil = p1tmp.tile([P, cw], f32, tag="sil")
                        nc.vector.tensor_mul(sil[:], ps1[ci][:], sig[:])
                        nc.vector.tensor_mul(it[:, c0:c0 + cw], sil[:],
                                             ps3[ci][:])
                else:
                    for ci, (c0, cw) in enumerate(chunks):
                        ps1s = psum_pool.tile([P, cw], f32, tag="ps1")
                        ps3s = psum_pool.tile([P, cw], f32, tag="ps3")
                        for hi in range(nh):
                            nc.tensor.matmul(
                                ps1s[:], wc[:, 0, hi, :],
                                xt_tiles[ci][:, hi, :],
                                start=(hi == 0), stop=(hi == nh - 1))
                        for hi in range(nh):
                            nc.tensor.matmul(
                                ps3s[:], wc[:, 1, hi, :],
                                xt_tiles[ci][:, hi, :],
                                start=(hi == 0), stop=(hi == nh - 1))
                        sig = p1tmp.tile([P, cw], f32, tag="sig")
                        nc.scalar.activation(sig[:], ps1s[:], AF.Sigmoid)
                        sil = p1tmp.tile([P, cw], f32, tag="sil")
                        nc.vector.tensor_mul(sil[:], ps1s[:], sig[:])
                        nc.vector.tensor_mul(it[:, c0:c0 + cw], sil[:],
                                             ps3s[:])

            # ---- phase 2: yT[ht, :] = (w2.T @ interT) * wgt ----
            # w2 columns stream on the ACT HWDGE ring; output on SWDGE
            # w2 streams on the SP ring: it naturally queues behind the wc
            # loads, so the prefetch cannot crowd out startup or stall the
            # ACT engine (which runs the phase-1 sigmoids).
            for ht in range(nh):
                w2c = w2_pool.tile([P, nf, P], dmm, tag="w2c")
                nc.sync.dma_start(w2c[:], w2_d[ht])
                ob = p2tmp.tile([P, C], f32, tag="ob")
                if nch <= 2:
                    po = [psum_pool.tile([P, cw], f32, tag="po", bufs=2,
                                         name=f"po_{ht}_{ci}")
                          for ci, (c0, cw) in enumerate(chunks)]
                    for fi in range(nf):
                        for ci in range(nch):
                            nc.tensor.matmul(
                                po[ci][:], w2c[:, fi, :],
                                inter_tiles[fi][:, chunks[ci][0]:
                                                chunks[ci][0] + chunks[ci][1]],
                                start=(fi == 0), stop=(fi == nf - 1))
                    for ci, (c0, cw) in enumerate(chunks):
                        nc.vector.tensor_mul(ob[:, c0:c0 + cw], po[ci][:],
                                             wgtb[:, c0:c0 + cw])
                else:
                    for ci, (c0, cw) in enumerate(chunks):
                        pos = psum_pool.tile([P, cw], f32, tag="po")
                        for fi in range(nf):
                            nc.tensor.matmul(
                                pos[:], w2c[:, fi, :],
                                inter_tiles[fi][:, c0:c0 + cw],
                                start=(fi == 0), stop=(fi == nf - 1))
                        nc.vector.tensor_mul(ob[:, c0:c0 + cw], pos[:],
                                             wgtb[:, c0:c0 + cw])
                nc.gpsimd.dma_start(y_d[ht * P:(ht + 1) * P, :], ob[:])

    nc.compile()
    return nc


def _get_nc(C, dt_mode):
    key = (C, dt_mode)
    if key not in _BUILD_CACHE:
        _BUILD_CACHE[key] = _build(C, dt_mode)
    return _BUILD_CACHE[key]


def _route(x, gate_w, top_k):
    """Host routing, matching the reference exactly:
    softmax(x @ gate_w.T) -> top-k (ties -> lower index) -> renormalize."""
    t = x.shape[0]
    logits = x.astype(np.float64) @ gate_w.astype(np.float64).T
    m = logits.max(axis=-1, keepdims=True)
    p = np.exp(logits - m)
    p /= p.sum(axis=-1, keepdims=True)
    idx = np.argsort(-p, axis=-1, kind="stable")[:, :top_k]          # [T, k]
    vals = np.take_along_axis(p, idx, axis=-1)
    vals = vals / vals.sum(axis=-1, keepdims=True)
    return idx, vals.astype(np.float32)


def _fake_device(in_maps):
    """Numpy stand-in for the device: consumes the exact tiled in_maps
    (validates host-side layouts end-to-end). Dev aid, off by default."""
    class R:
        exec_time_ns = None
        mean_exec_time_ns = None
        results = []
    res = R()
    for m in in_maps:
        C = m["xt"].shape[2]
        xs = m["xt"].transpose(1, 0, 2).reshape(H, C).T.astype(np.float32)
        w1e = m["w13t"][:, :, 0].transpose(0, 3, 2, 1).reshape(F, H).astype(
            np.float32)
        w3e = m["w13t"][:, :, 1].transpose(0, 3, 2, 1).reshape(F, H).astype(
            np.float32)
        w2e = m["w2t"].transpose(2, 1, 0, 3).reshape(F, H).astype(np.float32)
        wgt = m["wgtb"][0]
        h1 = xs @ w1e.T
        h3 = xs @ w3e.T
        inter = (h1 / (1 + np.exp(-h1))) * h3
        y = (inter @ w2e) * wgt[:, None]
        res.results.append({"yt": np.ascontiguousarray(y.T)})
    return res


def kernel(x, gate_w, w1, w2, w3, top_k):
    from concourse.bass_utils import run_bass_kernel_spmd

    x = np.ascontiguousarray(np.asarray(x, dtype=np.float32))
    gate_w = np.asarray(gate_w, dtype=np.float32)
    w1 = np.asarray(w1, dtype=np.float32)
    w2 = np.asarray(w2, dtype=np.float32)
    w3 = np.asarray(w3, dtype=np.float32)
    k = int(np.asarray(top_k))
    t, h = x.shape
    e = gate_w.shape[0]
    f = w1.shape[0] // e
    assert (h, f, e) == (H, F, E), (h, f, e)

    dt_mode = DT_MODE
    import ml_dtypes
    np_mm = {"bf16": ml_dtypes.bfloat16, "fp16": np.float16}.get(
        dt_mode, np.float32)

    idx, vals = _route(x, gate_w, k)                                  # [T, k]

    # token lists per expert
    tok_lists = []
    wgt_lists = []
    for ei in range(E):
        tok_i, slot_i = np.nonzero(idx == ei)
        tok_lists.append(tok_i.astype(np.int64))
        wgt_lists.append(vals[tok_i, slot_i].astype(np.float32))
    max_count = max(len(ti) for ti in tok_lists)
    C = max(256, ((max_count + 3) // 4) * 4)

    xmm = x.astype(np_mm)
    in_maps = []
    for ei in range(E):
        tok = tok_lists[ei]
        n = len(tok)
        xs = np.zeros((C, H), dtype=np_mm)
        xs[:n] = xmm[tok]
        # xt [P, NH, C]
        xt = np.ascontiguousarray(xs.T.reshape(NH, P, C).transpose(1, 0, 2))
        w1e = w1[ei * F:(ei + 1) * F].astype(np_mm)
        w3e = w3[ei * F:(ei + 1) * F].astype(np_mm)
        w2e = w2[ei * F:(ei + 1) * F].astype(np_mm)
        # w13t [NF, P, 2, NH, P]: [fi, p, m, n, j] = w{1,3}[fi*P+j, n*P+p]
        w13t = np.ascontiguousarray(np.stack(
            [w1e.reshape(NF, P, NH, P).transpose(0, 3, 2, 1),
             w3e.reshape(NF, P, NH, P).transpose(0, 3, 2, 1)], axis=2))
        # w2t [NH, P, NF, P]: [ht, p, fi, j] = w2[fi*P+p, ht*P+j]
        w2t = np.ascontiguousarray(
            w2e.reshape(NF, P, NH, P).transpose(2, 1, 0, 3))
        wgt = np.zeros(C, dtype=np.float32)
        wgt[:n] = wgt_lists[ei]
        wgtb = np.ascontiguousarray(
            np.broadcast_to(wgt, (P, C)).astype(np.float32))
        in_maps.append({
            "xt": xt, "w13t": w13t, "w2t": w2t, "wgtb": wgtb,
        })

    if os.environ.get("MOE_FAKE"):
        res = _fake_device(in_maps)
    else:
        nc = _get_nc(C, dt_mode)
        trace = bool(int(os.environ.get("MOE_TRACE", "0")))
        res = run_bass_kernel_spmd(nc, in_maps, core_ids=list(range(NCORES)),
                                   trace=trace)
    LAST_STATS.clear()
    LAST_STATS.update({
        "C": C,
        "dt_mode": dt_mode,
        "exec_time_ns": res.exec_time_ns,
        "mean_exec_time_ns": res.mean_exec_time_ns,
        "counts": [len(ti) for ti in tok_lists],
    })

    out = np.zeros((t, h), dtype=np.float32)
    for ei in range(E):
        n = len(tok_lists[ei])
        yt = res.results[ei]["yt"]                                    # [H, C]
        out[tok_lists[ei]] += yt[:, :n].T
    return out


# revision 30
# speedup vs baseline: 1.1790x; 1.0062x over previous
"""Block-sparse MoE (SwiGLU, top-k of 8 experts) on 8 Trainium2 NeuronCores.

Sharding: expert-parallel with host-side all-to-all token dispatch.
  - Routing (gate matmul + softmax + top-k, ~0.07% of total FLOPs) runs on
    the host; each token is dispatched to its top-k experts.
  - Core e runs the SwiGLU FFN of expert e over only the tokens routed to
    it (padded to a uniform capacity C), pre-scaled by the routing weight.
  - The host scatter-adds the per-expert outputs into the full [T, H] out.

Device kernel per core (capacity C tokens, H=2048, F=4096):
  phase 1: interT[f, c] = silu(w1 @ xT) * (w3 @ xT)   (PSUM-accumulated over H)
  phase 2: yT[h, c]     = (w2.T @ interT) * wgt[c]    (PSUM-accumulated over F)
Matmuls run as float32r (fp32 data, full PE rate at moving-dim >= 256).
All DRAM->SBUF transfers are host-pre-tiled so every DMA is contiguous.
"""

import math
import os

import numpy as np

H = 2048          # hidden dim
F = 4096          # ffn dim per expert
E = 8             # experts
NCORES = 8
P = 128           # partitions
NH = H // P       # 16 h-tiles
NF = F // P       # 32 f-tiles

DT_MODE = os.environ.get("MOE_DT", "fp16")   # fp16 | f32r | bf16 | f32

# populated by kernel() for test harness introspection
LAST_STATS = {}

_BUILD_CACHE = {}


def _chunks(total, cmax=512):
    """Split `total` into near-equal EVEN-width chunks each <= cmax
    (>=256 when total>=256; fp32r matmuls require an even moving dim)."""
    assert total % 2 == 0, total
    n = max(1, math.ceil(total / cmax))
    w0 = 2 * math.ceil(total / (2 * n))
    out = []
    pos = 0
    for i in range(n):
        w = min(w0, total - pos)
        out.append((pos, w))
        pos += w
    assert pos == total and all(w % 2 == 0 for _, w in out)
    return out


def _build(C, dt_mode, h=H, f=F):
    """Build + compile the per-core Bass program for capacity C."""
    import concourse.bacc as bacc
    import concourse.mybir as mybir
    from concourse import tile

    AF = mybir.ActivationFunctionType
    f32 = mybir.dt.float32
    if dt_mode == "bf16":
        dmm = mybir.dt.bfloat16    # dtype of matmul operands as stored
    elif dt_mode == "fp16":
        dmm = mybir.dt.float16
    elif dt_mode == "f32":
        dmm = f32
    else:
        # float32r: fp32 bytes, reduced-precision multiply at full PE rate.
        # The BIR verifier requires every producer of an fp32r matmul
        # operand to emit float32r, so the whole operand chain is f32r.
        dmm = mybir.dt.float32r

    nh, nf = h // P, f // P
    chunks = _chunks(C)

    def mm_ap(ap):
        return ap

    nc = bacc.Bacc("TRN2", target_bir_lowering=False, debug=False)

    # Host-pre-tiled DRAM layouts (every DMA below is fully contiguous):
    #   xt   [P, nh, C]        xt[p, n, c]        = x_tok[c, n*P + p]
    #   w13t [nf, P, 2, nh, P] w13t[fi, p, m, n, j] = w{1,3}[fi*P+j, n*P+p]
    #   w2t  [nh, P, nf, P]    w2t[ht, p, fi, j]  = w2[fi*P + p, ht*P + j]
    #   wgtb [P, C]            broadcast routing weights
    #   yt   [h, C]            output, yt[h, c]
    xt_d = nc.dram_tensor("xt", [P, nh, C], dmm, kind="ExternalInput").ap()
    w13_d = nc.dram_tensor("w13t", [nf, P, 2, nh, P], dmm,
                           kind="ExternalInput").ap()
    w2_d = nc.dram_tensor("w2t", [nh, P, nf, P], dmm, kind="ExternalInput").ap()
    wg_d = nc.dram_tensor("wgtb", [P, C], f32, kind="ExternalInput").ap()
    y_d = nc.dram_tensor("yt", [h, C], f32, kind="ExternalOutput").ap()

    with tile.TileContext(nc) as tc:
        with (
            tc.tile_pool(name="inter", bufs=1) as inter_pool,
            tc.tile_pool(name="psum", bufs=2, space="PSUM") as psum_pool,
            tc.tile_pool(name="misc", bufs=1) as misc_pool,
            tc.tile_pool(name="xtp", bufs=1) as xt_pool,
            tc.tile_pool(name="wcol", bufs=2) as wcol_pool,
            tc.tile_pool(name="p1tmp", bufs=2) as p1tmp,
            tc.tile_pool(name="w2col", bufs=2) as w2_pool,
            tc.tile_pool(name="p2tmp", bufs=2) as p2tmp,
        ):
            # xt split per chunk, spread across DMA rings, so the first
            # matmuls start as early as possible (a single big load added
            # a ~26 us PE startup stall)
            xt_rings = [nc.scalar]
            xt_tiles = []
            for ci, (c0, cw) in enumerate(chunks):
                xtc = xt_pool.tile([P, nh, cw], dmm, tag=f"xt{ci}",
                                   name=f"xt{ci}")
                xt_rings[ci % len(xt_rings)].dma_start(
                    xtc[:], xt_d[:, :, c0:c0 + cw])
                xt_tiles.append(xtc)

            wgtb = misc_pool.tile([P, C], f32, tag="wgtb")
            wgtb_dma = nc.gpsimd.dma_start(wgtb[:], wg_d[:])

            inter_tiles = []
            nch = len(chunks)

            # ---- phase 1: interT[f, :] = silu(w1 @ xT) * (w3 @ xT) ----
            # weight columns stream on the SP HWDGE ring (one 2 MB DMA/fi).
            # chunks are interleaved per h-tile so consecutive matmuls share
            # the same stationary operand.
            wc_dmas = []
            for fi in range(nf):
                wc = wcol_pool.tile([P, 2, nh, P], dmm, tag="wc")
                wc_dmas.append(nc.sync.dma_start(wc[:], w13_d[fi]))
                if fi == nf // 4:
                    # wgtb is needed only in phase 2; keep it off the
                    # HBM-saturated startup window. (gpsimd's queue is
                    # otherwise idle until the phase-2 output stores.)
                    tile.add_dep_helper(
                        wgtb_dma.ins, wc_dmas[-1].ins,
                        reason="delay wgtb load past kernel startup")
                it = inter_pool.tile([P, C], dmm, tag=f"inter{fi}",
                                     name=f"inter{fi}")
                inter_tiles.append(it)
                if nch <= 2:
                    # interleaved: consecutive matmuls share the stationary
                    ps1 = [psum_pool.tile([P, cw], f32, tag="ps1", bufs=3,
                                          name=f"ps1_{fi}_{ci}")
                           for ci, (c0, cw) in enumerate(chunks)]
                    ps3 = [psum_pool.tile([P, cw], f32, tag="ps3", bufs=3,
                                          name=f"ps3_{fi}_{ci}")
                           for ci, (c0, cw) in enumerate(chunks)]
                    for hi in range(nh):
                        for ci in range(nch):
                            nc.tensor.matmul(
                                ps1[ci][:], wc[:, 0, hi, :],
                                xt_tiles[ci][:, hi, :],
                                start=(hi == 0), stop=(hi == nh - 1))
                        for ci in range(nch):
                            nc.tensor.matmul(
                                ps3[ci][:], wc[:, 1, hi, :],
                                xt_tiles[ci][:, hi, :],
                                start=(hi == 0), stop=(hi == nh - 1))
                    for ci, (c0, cw) in enumerate(chunks):
                        # silu(a) = a * sigmoid(a)
                        sig = p1tmp.tile([P, cw], f32, tag="sig")
                        nc.scalar.activation(sig[:], ps1[ci][:], AF.Sigmoid)
                        sil = p1tmp.tile([P, cw], f32, tag="sil")
                        nc.vector.tensor_mul(sil[:], ps1[ci][:], sig[:])
                        nc.vector.tensor_mul(it[:, c0:c0 + cw], sil[:],
                                             ps3[ci][:])
                else:
                    for ci, (c0, cw) in enumerate(chunks):
                        ps1s = psum_pool.tile([P, cw], f32, tag="ps1")
                        ps3s = psum_pool.tile([P, cw], f32, tag="ps3")
                        for hi in range(nh):
                            nc.tensor.matmul(
                                ps1s[:], wc[:, 0, hi, :],
                                xt_tiles[ci][:, hi, :],
                                start=(hi == 0), stop=(hi == nh - 1))
                        for hi in range(nh):
                            nc.tensor.matmul(
                                ps3s[:], wc[:, 1, hi, :],
                                xt_tiles[ci][:, hi, :],
                                start=(hi == 0), stop=(hi == nh - 1))
                        sig = p1tmp.tile([P, cw], f32, tag="sig")
                        nc.scalar.activation(sig[:], ps1s[:], AF.Sigmoid)
                        sil = p1tmp.tile([P, cw], f32, tag="sil")
                        nc.vector.tensor_mul(sil[:], ps1s[:], sig[:])
                        nc.vector.tensor_mul(it[:, c0:c0 + cw], sil[:],
                                             ps3s[:])

            # ---- phase 2: yT[ht, :] = (w2.T @ interT) * wgt ----
            # w2 columns stream on the ACT HWDGE ring; output on SWDGE
            # w2 streams on the SP ring: it naturally queues behind the wc
            # loads, so the prefetch cannot crowd out startup or stall the
            # ACT engine (which runs the phase-1 sigmoids).
            for ht in range(nh):
                w2c = w2_pool.tile([P, nf, P], dmm, tag="w2c")
                nc.sync.dma_start(w2c[:], w2_d[ht])
                ob = p2tmp.tile([P, C], f32, tag="ob")
                if nch <= 2:
                    po = [psum_pool.tile([P, cw], f32, tag="po", bufs=2,
                                         name=f"po_{ht}_{ci}")
                          for ci, (c0, cw) in enumerate(chunks)]
                    for fi in range(nf):
                        for ci in range(nch):
                            nc.tensor.matmul(
                                po[ci][:], w2c[:, fi, :],
                                inter_tiles[fi][:, chunks[ci][0]:
                                                chunks[ci][0] + chunks[ci][1]],
                                start=(fi == 0), stop=(fi == nf - 1))
                    for ci, (c0, cw) in enumerate(chunks):
                        nc.vector.tensor_mul(ob[:, c0:c0 + cw], po[ci][:],
                                             wgtb[:, c0:c0 + cw])
                else:
                    for ci, (c0, cw) in enumerate(chunks):
                        pos = psum_pool.tile([P, cw], f32, tag="po")
                        for fi in range(nf):
                            nc.tensor.matmul(
                                pos[:], w2c[:, fi, :],
                                inter_tiles[fi][:, c0:c0 + cw],
                                start=(fi == 0), stop=(fi == nf - 1))
                        nc.vector.tensor_mul(ob[:, c0:c0 + cw], pos[:],
                                             wgtb[:, c0:c0 + cw])
                nc.gpsimd.dma_start(y_d[ht * P:(ht + 1) * P, :], ob[:])

    nc.compile()
    return nc


def _get_nc(C, dt_mode):
    key = (C, dt_mode)
    if key not in _BUILD_CACHE:
        _BUILD_CACHE[key] = _build(C, dt_mode)
    return _BUILD_CACHE[key]


def _route(x, gate_w, top_k):
    """Host routing, matching the reference exactly:
    softmax(x @ gate_w.T) -> top-k (ties -> lower index) -> renormalize."""
    t = x.shape[0]
    logits = x.astype(np.float64) @ gate_w.astype(np.float64).T
    m = logits.max(axis=-1, keepdims=True)
    p = np.exp(logits - m)
    p /= p.sum(axis=-1, keepdims=True)
    idx = np.argsort(-p, axis=-1, kind="stable")[:, :top_k]          # [T, k]
    vals = np.take_along_axis(p, idx, axis=-1)
    vals = vals / vals.sum(axis=-1, keepdims=True)
    return idx, vals.astype(np.float32)


def _fake_device(in_maps):
    """Numpy stand-in for the device: consumes the exact tiled in_maps
    (validates host-side layouts end-to-end). Dev aid, off by default."""
    class R:
        exec_time_ns = None
        mean_exec_time_ns = None
        results = []
    res = R()
    for m in in_maps:
        C = m["xt"].shape[2]
        xs = m["xt"].transpose(1, 0, 2).reshape(H, C).T.astype(np.float32)
        w1e = m["w13t"][:, :, 0].transpose(0, 3, 2, 1).reshape(F, H).astype(
            np.float32)
        w3e = m["w13t"][:, :, 1].transpose(0, 3, 2, 1).reshape(F, H).astype(
            np.float32)
        w2e = m["w2t"].transpose(2, 1, 0, 3).reshape(F, H).astype(np.float32)
        wgt = m["wgtb"][0]
        h1 = xs @ w1e.T
        h3 = xs @ w3e.T
        inter = (h1 / (1 + np.exp(-h1))) * h3
        y = (inter @ w2e) * wgt[:, None]
        res.results.append({"yt": np.ascontiguousarray(y.T)})
    return res


def kernel(x, gate_w, w1, w2, w3, top_k):
    from concourse.bass_utils import run_bass_kernel_spmd

    x = np.ascontiguousarray(np.asarray(x, dtype=np.float32))
    gate_w = np.asarray(gate_w, dtype=np.float32)
    w1 = np.asarray(w1, dtype=np.float32)
    w2 = np.asarray(w2, dtype=np.float32)
    w3 = np.asarray(w3, dtype=np.float32)
    k = int(np.asarray(top_k))
    t, h = x.shape
    e = gate_w.shape[0]
    f = w1.shape[0] // e
    assert (h, f, e) == (H, F, E), (h, f, e)

    dt_mode = DT_MODE
    import ml_dtypes
    np_mm = {"bf16": ml_dtypes.bfloat16, "fp16": np.float16}.get(
        dt_mode, np.float32)

    idx, vals = _route(x, gate_w, k)                                  # [T, k]

    # token lists per expert
    tok_lists = []
    wgt_lists = []
    for ei in range(E):
        tok_i, slot_i = np.nonzero(idx == ei)
        tok_lists.append(tok_i.astype(np.int64))
        wgt_lists.append(vals[tok_i, slot_i].astype(np.float32))
    max_count = max(len(ti) for ti in tok_lists)
    C = max(256, ((max_count + 3) // 4) * 4)

    xmm = x.astype(np_mm)
    in_maps = []
    for ei in range(E):
        tok = tok_lists[ei]
        n = len(tok)
        xs = np.zeros((C, H), dtype=np_mm)
        xs[:n] = xmm[tok]
        # xt [P, NH, C]
        xt = np.ascontiguousarray(xs.T.reshape(NH, P, C).transpose(1, 0, 2))
        w1e = w1[ei * F:(ei + 1) * F].astype(np_mm)
        w3e = w3[ei * F:(ei + 1) * F].astype(np_mm)
        w2e = w2[ei * F:(ei + 1) * F].astype(np_mm)
        # w13t [NF, P, 2, NH, P]: [fi, p, m, n, j] = w{1,3}[fi*P+j, n*P+p]
        w13t = np.ascontiguousarray(np.stack(
            [w1e.reshape(NF, P, NH, P).transpose(0, 3, 2, 1),
             w3e.reshape(NF, P, NH, P).transpose(0, 3, 2, 1)], axis=2))
        # w2t [NH, P, NF, P]: [ht, p, fi, j] = w2[fi*P+p, ht*P+j]
        w2t = np.ascontiguousarray(
            w2e.reshape(NF, P, NH, P).transpose(2, 1, 0, 3))
        wgt = np.zeros(C, dtype=np.float32)
        wgt[:n] = wgt_lists[ei]
        wgtb = np.ascontiguousarray(
            np.broadcast_to(wgt, (P, C)).astype(np.float32))
        in_maps.append({
            "xt": xt, "w13t": w13t, "w2t": w2t, "wgtb": wgtb,
        })

    if os.environ.get("MOE_FAKE"):
        res = _fake_device(in_maps)
    else:
        nc = _get_nc(C, dt_mode)
        trace = bool(int(os.environ.get("MOE_TRACE", "0")))
        res = run_bass_kernel_spmd(nc, in_maps, core_ids=list(range(NCORES)),
                                   trace=trace)
    LAST_STATS.clear()
    LAST_STATS.update({
        "C": C,
        "dt_mode": dt_mode,
        "exec_time_ns": res.exec_time_ns,
        "mean_exec_time_ns": res.mean_exec_time_ns,
        "counts": [len(ti) for ti in tok_lists],
    })

    out = np.zeros((t, h), dtype=np.float32)
    for ei in range(E):
        n = len(tok_lists[ei])
        yt = res.results[ei]["yt"]                                    # [H, C]
        out[tok_lists[ei]] += yt[:, :n].T
    return out
